# revision 1
# baseline (speedup 1.0000x reference)
"""Causal masked single-head attention [B=4, N=4096, D=768] on 8 trn2 cores.

Sharding: 2 cores per batch element; each core owns 8 query blocks of 256
rows, round-robin over the 16 blocks (core parity c gets blocks 2j+c) so
causal work is balanced. All 8 cores run one identical instruction stream;
per-core differences live entirely in input data (query gather order and
the additive causal masks for the 4 tail key tiles of each q-block).

On-device (per core, matmuls bf16 -> f32 PSUM): K^T/V/Q^T projections
(transposed layouts so attention needs no on-device transposes), V carries
a ones-column so the softmax denominator falls out of the same PSUM
accumulation as P@V; exp on ScalarE; normalize via DVE reciprocal.
"""

import math
import sys

sys.path.insert(0, "/opt/trn_rl_repo")

import numpy as np
import ml_dtypes

import concourse.bass as bass
import concourse.bacc as bacc
import concourse.mybir as mybir
import concourse.tile as tile
from concourse.bass_utils import run_bass_kernel_spmd

F32 = mybir.dt.float32
BF16 = mybir.dt.bfloat16
NEG = -1.0e5

# ---------------------------------------------------------------------------
# Kernel builder
# ---------------------------------------------------------------------------


class Cfg:
    def __init__(self, D=768, N=4096, QB=256):
        assert D % 128 == 0 and N % 512 == 0 and QB == 256
        self.D = D
        self.N = N
        self.QB = QB
        self.QC = N // 2
        self.NDT = D // 128
        self.NOT = D // 128
        self.OH = D // 2
        self.NKB = N // 512
        self.NKT = N // 128
        self.NQB = self.QC // QB
        self.scale = 1.0 / math.sqrt(D)


def build_kernel(cfg: Cfg, repeat: int = 1) -> bass.Bass:
    import contextlib
    D, N, QB = cfg.D, cfg.N, cfg.QB
    nc = bacc.Bacc("TRN2")

    xT = nc.dram_tensor("xT", [D, N], BF16, kind="ExternalInput")
    xqT = nc.dram_tensor("xqT", [D, cfg.QC], BF16, kind="ExternalInput")
    wqT = nc.dram_tensor("wqT", [D, D], BF16, kind="ExternalInput")
    wkT = nc.dram_tensor("wkT", [D, D], BF16, kind="ExternalInput")
    wvT = nc.dram_tensor("wvT", [D, D], BF16, kind="ExternalInput")
    mask4 = nc.dram_tensor("mask4", [128, 4 * QB], F32, kind="ExternalInput")
    out = nc.dram_tensor("out", [cfg.QC, D], F32, kind="ExternalOutput")

    with tile.TileContext(nc) as tc:
        with (
            tc.tile_pool(name="persist", bufs=1) as persist,
            tc.tile_pool(name="xstream", bufs=2) as xstream,
            tc.tile_pool(name="work", bufs=3) as work,
            tc.tile_pool(name="sc", bufs=2, space="PSUM") as scp,
            tc.tile_pool(name="ops", bufs=6, space="PSUM") as opsp,
        ):
            wq_t = [persist.tile([128, D], BF16, tag=f"wq{dt}", name=f"wq{dt}")
                    for dt in range(cfg.NDT)]
            wk_t = [persist.tile([128, D], BF16, tag=f"wk{dt}", name=f"wk{dt}")
                    for dt in range(cfg.NDT)]
            wv_t = [persist.tile([128, D], BF16, tag=f"wv{dt}", name=f"wv{dt}")
                    for dt in range(cfg.NDT)]
            KT = [persist.tile([128, N], BF16, tag=f"KT{ot}", name=f"KT{ot}")
                  for ot in range(cfg.NOT)]
            QT = [persist.tile([128, cfg.QC], BF16, tag=f"QT{ot}", name=f"QT{ot}")
                  for ot in range(cfg.NOT)]
            V = [persist.tile([128, D + 1], BF16, tag=f"V{kt}", name=f"V{kt}")
                 for kt in range(cfg.NKT)]
            msk = persist.tile([128, 4 * QB], F32, tag="msk")

            rep_ctx = tc.For_i(0, repeat, 1) if repeat > 1 else contextlib.nullcontext()
            with rep_ctx:
                for dt in range(cfg.NDT):
                    nc.sync.dma_start(wq_t[dt][:], wqT[128 * dt:128 * (dt + 1), :])
                    nc.sync.dma_start(wk_t[dt][:], wkT[128 * dt:128 * (dt + 1), :])
                    nc.sync.dma_start(wv_t[dt][:], wvT[128 * dt:128 * (dt + 1), :])
                nc.sync.dma_start(msk[:], mask4[:])
                for kt in range(cfg.NKT):
                    nc.gpsimd.memset(V[kt][:, D:D + 1], 1.0)

                # K and V projections, streamed over 512-key col blocks
                for kb in range(cfg.NKB):
                    xb = []
                    for dt in range(cfg.NDT):
                        t = xstream.tile([128, 512], BF16, tag=f"xb{dt}", name=f"xb{dt}")
                        nc.sync.dma_start(t[:], xT[128 * dt:128 * (dt + 1),
                                                   512 * kb:512 * (kb + 1)])
                        xb.append(t)
                    for ot in range(cfg.NOT):
                        ps = scp.tile([128, 512], F32, tag="sc", name="pj")
                        for dt in range(cfg.NDT):
                            nc.tensor.matmul(
                                ps[:], wk_t[dt][:, 128 * ot:128 * (ot + 1)], xb[dt][:],
                                start=(dt == 0), stop=(dt == cfg.NDT - 1))
                        nc.vector.tensor_copy(KT[ot][:, 512 * kb:512 * (kb + 1)], ps[:])
                    for kl in range(4):
                        kt = 4 * kb + kl
                        for oh in range(2):
                            ps = scp.tile([128, 512], F32, tag="sc", name="pj")
                            for dt in range(cfg.NDT):
                                nc.tensor.matmul(
                                    ps[:, 0:cfg.OH],
                                    xb[dt][:, 128 * kl:128 * (kl + 1)],
                                    wv_t[dt][:, cfg.OH * oh:cfg.OH * (oh + 1)],
                                    start=(dt == 0), stop=(dt == cfg.NDT - 1))
                            nc.vector.tensor_copy(
                                V[kt][:, cfg.OH * oh:cfg.OH * (oh + 1)], ps[:, 0:cfg.OH])

                # Q projection
                for qb in range(cfg.QC // 512):
                    xq = []
                    for dt in range(cfg.NDT):
                        t = xstream.tile([128, 512], BF16, tag=f"xb{dt}", name=f"xb{dt}")
                        nc.sync.dma_start(t[:], xqT[128 * dt:128 * (dt + 1),
                                                    512 * qb:512 * (qb + 1)])
                        xq.append(t)
                    for ot in range(cfg.NOT):
                        ps = scp.tile([128, 512], F32, tag="sc", name="pj")
                        for dt in range(cfg.NDT):
                            nc.tensor.matmul(
                                ps[:], wq_t[dt][:, 128 * ot:128 * (ot + 1)], xq[dt][:],
                                start=(dt == 0), stop=(dt == cfg.NDT - 1))
                        nc.vector.tensor_copy(QT[ot][:, 512 * qb:512 * (qb + 1)], ps[:])

                # attention over q blocks
                for j in range(cfg.NQB):
                    nkt = 4 * j + 4
                    ops = [[opsp.tile([128, cfg.OH + 1], F32, tag="ops", name="ops")
                            for _ in range(2)] for _ in range(2)]
                    for kt in range(nkt):
                        st = scp.tile([128, QB], F32, tag="sc", name="st")
                        for ot in range(cfg.NOT):
                            nc.tensor.matmul(
                                st[:], KT[ot][:, 128 * kt:128 * (kt + 1)],
                                QT[ot][:, QB * j:QB * (j + 1)],
                                start=(ot == 0), stop=(ot == cfg.NOT - 1))
                        mi = kt - (nkt - 4)
                        if mi >= 0:
                            nc.vector.tensor_add(st[:], st[:],
                                                 msk[:, QB * mi:QB * (mi + 1)])
                        pt = work.tile([128, QB], BF16, tag="pt", name="pt")
                        nc.scalar.activation(pt[:], st[:],
                                             mybir.ActivationFunctionType.Exp,
                                             scale=cfg.scale)
                        for qh in range(2):
                            for oh in range(2):
                                w = cfg.OH + (1 if oh == 1 else 0)
                                nc.tensor.matmul(
                                    ops[qh][oh][:, 0:w],
                                    pt[:, 128 * qh:128 * (qh + 1)],
                                    V[kt][:, cfg.OH * oh:cfg.OH * oh + w],
                                    start=(kt == 0), stop=(kt == nkt - 1))
                    for qh in range(2):
                        rec = work.tile([128, 1], F32, tag="rec", name="rec")
                        nc.vector.reciprocal(rec[:], ops[qh][1][:, cfg.OH:cfg.OH + 1])
                        osb = work.tile([128, D], F32, tag="osb", name="osb")
                        nc.vector.tensor_scalar_mul(osb[:, 0:cfg.OH],
                                                    ops[qh][0][:, 0:cfg.OH], rec[:])
                        nc.vector.tensor_scalar_mul(osb[:, cfg.OH:D],
                                                    ops[qh][1][:, 0:cfg.OH], rec[:])
                        nc.sync.dma_start(
                            out[QB * j + 128 * qh:QB * j + 128 * (qh + 1), :], osb[:])
    nc.compile()
    return nc


# ---------------------------------------------------------------------------
# Host-side sharding / gather
# ---------------------------------------------------------------------------


def make_masks(QB: int, parity: int) -> np.ndarray:
    kk = np.arange(128)[:, None]
    qq = np.arange(QB)[None, :]
    tri0 = np.where(kk <= qq, 0.0, NEG).astype(np.float32)
    tri1 = np.where(kk + 128 <= qq, 0.0, NEG).astype(np.float32)
    zero = np.zeros((128, QB), np.float32)
    full = np.full((128, QB), NEG, np.float32)
    blocks = [tri0, tri1, full, full] if parity == 0 else [zero, zero, tri0, tri1]
    return np.concatenate(blocks, axis=1)


def core_inputs(cfg: Cfg, x_b: np.ndarray, WqT, WkT, WvT, parity: int) -> dict:
    bf = ml_dtypes.bfloat16
    QB = cfg.QB
    xT = np.ascontiguousarray(x_b.T).astype(bf)
    cols = []
    for j in range(cfg.NQB):
        gb = 2 * j + parity
        cols.append(x_b[QB * gb:QB * (gb + 1), :].T)
    xqT = np.ascontiguousarray(np.concatenate(cols, axis=1)).astype(bf)
    return {
        "xT": xT,
        "xqT": xqT,
        "wqT": WqT,
        "wkT": WkT,
        "wvT": WvT,
        "mask4": make_masks(QB, parity),
    }


def scatter_output(cfg: Cfg, out_core: np.ndarray, parity: int,
                   dst: np.ndarray) -> None:
    QB = cfg.QB
    for j in range(cfg.NQB):
        gb = 2 * j + parity
        dst[QB * gb:QB * (gb + 1), :] = out_core[QB * j:QB * (j + 1), :]


def build_in_maps(cfg: Cfg, input_batch, Wq, Wk, Wv):
    bf = ml_dtypes.bfloat16
    x = np.asarray(input_batch, dtype=np.float32)
    WqT = np.ascontiguousarray(np.asarray(Wq, np.float32).T).astype(bf)
    WkT = np.ascontiguousarray(np.asarray(Wk, np.float32).T).astype(bf)
    WvT = np.ascontiguousarray(np.asarray(Wv, np.float32).T).astype(bf)
    return [core_inputs(cfg, x[c // 2], WqT, WkT, WvT, c % 2) for c in range(8)]


_CACHE: dict = {}


def _get_nc(cfg: Cfg) -> bass.Bass:
    if "nc" not in _CACHE:
        _CACHE["nc"] = build_kernel(cfg)
    return _CACHE["nc"]


def kernel(input_batch, Wq, Wk, Wv):
    cfg = Cfg()
    nc = _get_nc(cfg)
    in_maps = build_in_maps(cfg, input_batch, Wq, Wk, Wv)
    res = run_bass_kernel_spmd(nc, in_maps, core_ids=list(range(8)))
    B = np.asarray(input_batch).shape[0]
    out = np.empty((B, cfg.N, cfg.D), np.float32)
    for c in range(2 * B):
        scatter_output(cfg, res.results[c]["out"], c % 2, out[c // 2])
    return out

